# revision 3
# baseline (speedup 1.0000x reference)
"""Trainium2 Bass kernel for nn_Decoder (NeRF-style 9-layer MLP, Softplus(beta=100)).

Strategy (pure data parallel over 8 cores, feature-major layout):
  - activations live in SBUF as z_l = 100 * y_l  (softplus "raw" outputs), shape
    [features<=100 partitions, points free-dim]; weights are stationary lhsT.
  - per layer: 4x fp32 matmul (N=512 each) into one PSUM tile [100, 2048],
    ACT: e = Exp(psum + 100*b)   (== e^u, +inf for u > 88.7 -- handled below)
    ACT: l = Ln(e + 1)           (== softplus(u) for moderate u)
    DVE: z = min(l, max(psum + 100*b, 85))   (single fused custom op; exact in
         fp32: for u<=85 picks l, for u>85 picks u, where softplus(u)==u in fp32)
  - skip connection (layer 4) handled by DMAing the raw input into partitions
    98:100 of the layer-3 output tile; layer-4 weights columns are scaled to match.
  - layer 8 (100->1, no activation): matmul + DVE bias add, DMA out.
The exp/ln pair lives in one ACT table set (natural_log_exp_and_others): no
table switching. All matmuls fp32 (exact).
"""

import numpy as np

import concourse.bass as bass
import concourse.tile as tile
from concourse import bacc, mybir
from concourse import bass_utils
from concourse.bass_interp import get_hw_module

F32 = mybir.dt.float32
F32R = mybir.dt.float32r
ACTF = mybir.ActivationFunctionType

N_CORES = 8
N_TOTAL = 1048576
P = N_TOTAL // N_CORES          # 131072 points per core
T = 2048                        # supertile (points per ACT instruction; 4 PSUM banks)
NT = P // T                     # supertiles per core
DIMS = [2, 100, 100, 100, 98, 100, 100, 100, 100, 1]

_SOFTPLUS_FIN = None


def _get_softplus_fin():
    """Register (once) the fused custom-DVE op:
        out = min(in0, max(in1 + s0, s1))
    in0 = Ln(e+1) tile (SBUF), in1 = psum (PSUM), s0 = +100*b [P,1], s1 = 85.0
    """
    global _SOFTPLUS_FIN
    if _SOFTPLUS_FIN is not None:
        return _SOFTPLUS_FIN
    from concourse import dve_ops
    from concourse.dve_spec import Spec, Src0, Src1, C0, C1, lower, maxx, minn, _has_src1
    from concourse.dve_uop import DveOpSpec
    from concourse.dve_table_gen import dve_ver_for

    name = "SOFTPLUS_FIN_ANT"
    # u = in1 + s0;  z = max(min(in0, max(u, s1)), u)
    # With s1=40: for u<=40 picks in0 (= Ln(e^u + 1), accurate there); for u>40
    # softplus(u) == u in fp32, and the outer max(_, u) also discards any
    # garbage the Ln table emits for huge inputs (x > 2.3e19) of either sign.
    _uu = Src1 + C0
    spec = Spec(
        body=maxx(minn(Src0, maxx(_uu, C1)), _uu),
        reference=lambda in0, in1, s0, s1, imm2: np.maximum(
            np.minimum(in0, np.maximum(in1.astype(np.float32) + s0, s1)),
            in1.astype(np.float32) + s0,
        ),
    )
    op = dve_ops.DveOp(name, spec, subdim=False, uops_sha={})
    dve_ops.OPS.append(op)
    dve_ops.CUSTOM_DVE_SPECS[name] = spec
    dve_ops._SUB_OPCODE_FOR_NAME[name] = (
        dve_ops._CUSTOM_DVE_ROW_BASE + len(dve_ops.OPS) - 1
    )
    assert dve_ops._SUB_OPCODE_FOR_NAME[name] < 0x20
    for ver in ("v3", "v4"):
        uops = lower(spec, ver=ver)
        tmp = DveOpSpec(
            name=name,
            opcode=dve_ops.get_dve_sub_opcode(name),
            uops=uops,
            rd1_en=_has_src1(spec),
        )
        op.uops_sha[ver] = tmp.sha(ver)
    _SOFTPLUS_FIN = op
    return op


def _build_program():
    sp_fin = _get_softplus_fin()
    nc = bacc.Bacc(
        "TRN2",
        target_bir_lowering=False,
        debug=False,
        enable_asserts=False,
        num_devices=N_CORES,
    )

    # DRAM I/O (per core)
    xt_d = nc.dram_tensor("xt", [2, P], F32, kind="ExternalInput")
    lhsT_d = []
    bias_d = []
    for l in range(9):
        in_dim = 100 if l == 4 else DIMS[l]
        out_dim = DIMS[l + 1]
        lhsT_d.append(
            nc.dram_tensor(f"lhsT{l}", [in_dim, out_dim], F32, kind="ExternalInput")
        )
        if l < 8:
            bias_d.append(
                nc.dram_tensor(f"bias{l}", [out_dim, 1], F32, kind="ExternalInput")
            )
    b8_d = nc.dram_tensor("b8", [1, 1], F32, kind="ExternalInput")
    y_d = nc.dram_tensor("y", [1, P], F32, kind="ExternalOutput")

    with tile.TileContext(nc) as tc:
        with (
            tc.tile_pool(name="wpool", bufs=1) as wpool,
            tc.tile_pool(name="xpool", bufs=3) as xpool,
            tc.tile_pool(name="psum", bufs=2, space="PSUM") as pspool,
            tc.tile_pool(name="epool", bufs=2) as epool,
            tc.tile_pool(name="lpool", bufs=2) as lpool,
            tc.tile_pool(name="mpool", bufs=3) as mpool,
            tc.tile_pool(name="opool", bufs=2) as opool,
        ):
            # --- preload weights/biases ---
            wts = []
            bts = []
            for l in range(9):
                in_dim = 100 if l == 4 else DIMS[l]
                out_dim = DIMS[l + 1]
                wt = wpool.tile([in_dim, out_dim], F32, tag=f"w{l}")
                nc.sync.dma_start(wt[:], lhsT_d[l].ap())
                wts.append(wt)
                if l < 8:
                    bt = wpool.tile([out_dim, 1], F32, tag=f"b{l}")
                    nc.sync.dma_start(bt[:], bias_d[l].ap())
                    bts.append(bt)
            b8t = wpool.tile([1, 1], F32, tag="b8")
            nc.sync.dma_start(b8t[:], b8_d.ap())

            # --- main loop over supertiles ---
            for t in range(NT):
                sl = bass.ts(t, T)
                xt = xpool.tile([2, T], F32, tag="xt")
                nc.sync.dma_start(xt[:], xt_d.ap()[:, sl])

                prev = xt  # rhs of layer 0
                for l in range(9):
                    in_dim = 100 if l == 4 else DIMS[l]
                    out_dim = DIMS[l + 1]
                    ps = pspool.tile([100, T], F32, tag="ps")
                    for j in range(T // 512):
                        js = bass.ts(j, 512)
                        nc.tensor.matmul(
                            ps[0:out_dim, js],
                            wts[l][:],
                            prev[0:in_dim, js],
                            start=True,
                            stop=True,
                        )
                    if l == 8:
                        out_t = opool.tile([1, T], F32, tag="out")
                        nc.vector.tensor_scalar_add(
                            out_t[:], ps[0:1, :], b8t[0:1, 0:1]
                        )
                        nc.sync.dma_start(y_d.ap()[:, sl], out_t[:])
                        break
                    e = epool.tile([100, T], F32, tag="e")
                    nc.scalar.activation(
                        e[0:out_dim, :], ps[0:out_dim, :], ACTF.Exp,
                        bias=bts[l][:, 0:1], scale=1.0,
                    )
                    lt = lpool.tile([100, T], F32, tag="l")
                    nc.scalar.activation(
                        lt[0:out_dim, :], e[0:out_dim, :], ACTF.Ln, bias=1.0
                    )
                    m = mpool.tile([100, T], F32, tag="m")
                    if l == 3:
                        # skip connection: raw input occupies partitions 98:100
                        nc.sync.dma_start(m[98:100, :], xt_d.ap()[:, sl])
                    nc.vector._custom_dve(
                        sp_fin,
                        out=m[0:out_dim, :],
                        in0=lt[0:out_dim, :],
                        in1=ps[0:out_dim, :],
                        s0=bts[l][:, 0:1],
                        s1=40.0,
                    )
                    prev = m

    nc.compile()
    nc.m = get_hw_module(nc.m)
    return nc


def _transform_weights(inputs):
    """Host-side weight/bias transform -> per-program DRAM tensors (shared
    across cores)."""
    W = [np.asarray(inputs[f"W{l}"], dtype=np.float32) for l in range(9)]
    b = [np.asarray(inputs[f"b{l}"], dtype=np.float32) for l in range(9)]
    t = {}
    t["lhsT0"] = np.ascontiguousarray((100.0 * W[0]).T)
    for l in (1, 2, 3, 5, 6, 7):
        t[f"lhsT{l}"] = np.ascontiguousarray(W[l].T)
    t["lhsT4"] = np.ascontiguousarray(
        np.concatenate([W[4][:, 2:].T, (100.0 * W[4][:, :2]).T], axis=0)
    )
    t["lhsT8"] = np.ascontiguousarray(W[8].T / 100.0)
    for l in range(8):
        t[f"bias{l}"] = np.ascontiguousarray((100.0 * b[l]).reshape(-1, 1))
    t["b8"] = np.ascontiguousarray(b[8].reshape(1, 1))
    return t


_NC_CACHE = None


def kernel(**inputs) -> np.ndarray:
    global _NC_CACHE
    if _NC_CACHE is None:
        _NC_CACHE = _build_program()
    nc = _NC_CACHE

    x = np.asarray(inputs["input"], dtype=np.float32)
    assert x.shape == (N_TOTAL, 2)
    shared = _transform_weights(inputs)

    in_maps = []
    for c in range(N_CORES):
        m = dict(shared)
        m["xt"] = np.ascontiguousarray(x[c * P : (c + 1) * P].T)
        in_maps.append(m)

    res = bass_utils.run_bass_kernel_spmd(nc, in_maps, core_ids=list(range(N_CORES)))
    y = np.concatenate([res.results[c]["y"][0] for c in range(N_CORES)])
    return y.reshape(N_TOTAL, 1).astype(np.float32)


# revision 9
# speedup vs baseline: 4.0879x; 4.0879x over previous
"""Trainium2 Bass kernel for nn_Decoder (NeRF-style 9-layer MLP, Softplus(beta=100)).

Strategy (pure data parallel over 8 cores, feature-major layout):
  - activations live in SBUF as z_l = 100 * y_l  (softplus "raw" outputs), shape
    [features<=100 partitions, points free-dim]; weights are stationary lhsT.
  - per layer: 4x fp32 matmul (N=512 each) into one PSUM tile [100, 2048],
    ACT: e = Exp(psum + 100*b)   (== e^u, +inf for u > 88.7 -- handled below)
    ACT: l = Ln(e + 1)           (== softplus(u) for moderate u)
    DVE: z = min(l, max(psum + 100*b, 85))   (single fused custom op; exact in
         fp32: for u<=85 picks l, for u>85 picks u, where softplus(u)==u in fp32)
  - skip connection (layer 4) handled by DMAing the raw input into partitions
    98:100 of the layer-3 output tile; layer-4 weights columns are scaled to match.
  - layer 8 (100->1, no activation): matmul + DVE bias add, DMA out.
The exp/ln pair lives in one ACT table set (natural_log_exp_and_others): no
table switching. All matmuls fp32 (exact).
"""

import numpy as np

import concourse.bass as bass
import concourse.tile as tile
from concourse import bacc, mybir
from concourse import bass_utils
from concourse.bass_interp import get_hw_module

F32 = mybir.dt.float32
F32R = mybir.dt.float32r
ACTF = mybir.ActivationFunctionType

N_CORES = 8
N_TOTAL = 1048576
P = N_TOTAL // N_CORES          # 131072 points per core
T = 2048                        # supertile (points per ACT instruction; 4 PSUM banks)
NT = P // T                     # supertiles per core
DIMS = [2, 100, 100, 100, 98, 100, 100, 100, 100, 1]

_SOFTPLUS_FIN = None


def _get_softplus_fin():
    """Register (once) the fused custom-DVE op:
        out = min(in0, max(in1 + s0, s1))
    in0 = Ln(e+1) tile (SBUF), in1 = psum (PSUM), s0 = +100*b [P,1], s1 = 85.0
    """
    global _SOFTPLUS_FIN
    if _SOFTPLUS_FIN is not None:
        return _SOFTPLUS_FIN
    from concourse import dve_ops
    from concourse.dve_spec import Spec, Src0, Src1, C0, C1, lower, maxx, minn, _has_src1
    from concourse.dve_uop import DveOpSpec
    from concourse.dve_table_gen import dve_ver_for

    name = "SOFTPLUS_FIN_ANT"
    # u = in1 + s0;  z = max(min(in0, max(u, s1)), u)
    # With s1=40: for u<=40 picks in0 (= Ln(e^u + 1), accurate there); for u>40
    # softplus(u) == u in fp32, and the outer max(_, u) also discards any
    # garbage the Ln table emits for huge inputs (x > 2.3e19) of either sign.
    _uu = Src1 + C0
    spec = Spec(
        body=maxx(minn(Src0, maxx(_uu, C1)), _uu),
        reference=lambda in0, in1, s0, s1, imm2: np.maximum(
            np.minimum(in0, np.maximum(in1.astype(np.float32) + s0, s1)),
            in1.astype(np.float32) + s0,
        ),
    )
    op = dve_ops.DveOp(name, spec, subdim=False, uops_sha={})
    dve_ops.OPS.append(op)
    dve_ops.CUSTOM_DVE_SPECS[name] = spec
    dve_ops._SUB_OPCODE_FOR_NAME[name] = (
        dve_ops._CUSTOM_DVE_ROW_BASE + len(dve_ops.OPS) - 1
    )
    assert dve_ops._SUB_OPCODE_FOR_NAME[name] < 0x20
    for ver in ("v3", "v4"):
        uops = lower(spec, ver=ver)
        tmp = DveOpSpec(
            name=name,
            opcode=dve_ops.get_dve_sub_opcode(name),
            uops=uops,
            rd1_en=_has_src1(spec),
        )
        op.uops_sha[ver] = tmp.sha(ver)
    _SOFTPLUS_FIN = op
    return op


_TABLES_PATCHED = False


def _patch_act_tables():
    """Make natural_log_exp_and_others the only table set advertising Exp/Ln,
    so the table-load placement pass keeps one set loaded for the whole kernel
    instead of thrashing between exp_and_others and natural_log (~1024 reloads,
    ~1.3 ms). Set positions are preserved (position == act_func_set_id)."""
    global _TABLES_PATCHED
    if _TABLES_PATCHED:
        return
    import concourse.hw_specs as hw_specs
    import concourse.bacc as bacc_mod

    orig = hw_specs.get_activation_tables
    EXP = ACTF.Exp
    LN = ACTF.Ln

    def patched(module_arch):
        tables = {k: set(v) for k, v in orig(module_arch).items()}
        for name, funcs in tables.items():
            if name != "natural_log_exp_and_others":
                funcs.discard(EXP)
                funcs.discard(LN)
        return tables

    hw_specs.get_activation_tables = patched
    bacc_mod.get_activation_tables = patched
    _TABLES_PATCHED = True


def _build_program(T=1024, psum_bufs=4, ebufs=8, lbufs=8, mbufs=10, use_f32r=True, GRP=4, xbufs=6):
    NT = P // T
    _patch_act_tables()
    sp_fin = _get_softplus_fin()
    MMDT = F32R if use_f32r else F32
    nc = bacc.Bacc(
        "TRN2",
        target_bir_lowering=False,
        debug=False,
        enable_asserts=False,
        num_devices=N_CORES,
    )

    # DRAM I/O (per core)
    xt_d = nc.dram_tensor("xt", [2, P], F32, kind="ExternalInput")
    lhsT_d = []
    bias_d = []
    for l in range(9):
        in_dim = 100 if l == 4 else DIMS[l]
        out_dim = DIMS[l + 1]
        lhsT_d.append(
            nc.dram_tensor(f"lhsT{l}", [in_dim, out_dim], F32, kind="ExternalInput")
        )
        if l < 8:
            bias_d.append(
                nc.dram_tensor(f"bias{l}", [out_dim, 1], F32, kind="ExternalInput")
            )
    b8_d = nc.dram_tensor("b8", [1, 1], F32, kind="ExternalInput")
    y_d = nc.dram_tensor("y", [1, P], F32, kind="ExternalOutput")

    with tile.TileContext(nc) as tc:
        with (
            tc.tile_pool(name="wpool", bufs=1) as wpool,
            tc.tile_pool(name="xpool", bufs=xbufs) as xpool,
            tc.tile_pool(name="psum", bufs=psum_bufs, space="PSUM") as pspool,
            tc.tile_pool(name="epool", bufs=ebufs) as epool,
            tc.tile_pool(name="lpool", bufs=lbufs) as lpool,
            tc.tile_pool(name="mpool", bufs=mbufs) as mpool,
            tc.tile_pool(name="opool", bufs=2) as opool,
        ):
            # --- preload weights/biases ---
            wts = []
            bts = []
            for l in range(9):
                in_dim = 100 if l == 4 else DIMS[l]
                out_dim = DIMS[l + 1]
                wt = wpool.tile([in_dim, out_dim], MMDT, tag=f"w{l}")
                nc.sync.dma_start(wt[:], lhsT_d[l].ap().bitcast(MMDT))
                wts.append(wt)
                if l < 8:
                    bt = wpool.tile([out_dim, 1], F32, tag=f"b{l}")
                    nc.sync.dma_start(bt[:], bias_d[l].ap())
                    bts.append(bt)
            b8t = wpool.tile([1, 1], F32, tag="b8")
            nc.sync.dma_start(b8t[:], b8_d.ap())

            # --- main loop: software-pipeline GRP supertiles layer-by-layer
            # (interleaved emission so each engine's in-order stream ping-pongs
            # between independent supertiles instead of head-of-line blocking
            # on the serial per-supertile chain) ---
            assert NT % GRP == 0
            for g in range(NT // GRP):
                ts_ids = [g * GRP + i for i in range(GRP)]
                sls = [bass.ts(t, T) for t in ts_ids]
                xts = []
                for sl in sls:
                    xt = xpool.tile([2, T], MMDT, tag="xt")
                    nc.sync.dma_start(xt[:], xt_d.ap()[:, sl].bitcast(MMDT))
                    xts.append(xt)
                prevs = list(xts)
                for l in range(9):
                    in_dim = 100 if l == 4 else DIMS[l]
                    out_dim = DIMS[l + 1]
                    pss = []
                    for i in range(GRP):
                        ps = pspool.tile([100, T], F32, tag="ps")
                        for j in range(T // 512):
                            js = bass.ts(j, 512)
                            nc.tensor.matmul(
                                ps[0:out_dim, js],
                                wts[l][:],
                                prevs[i][0:in_dim, js],
                                start=True,
                                stop=True,
                            )
                        pss.append(ps)
                    if l == 8:
                        for i in range(GRP):
                            out_t = opool.tile([1, T], F32, tag="out")
                            nc.vector.tensor_scalar_add(
                                out_t[:], pss[i][0:1, :], b8t[0:1, 0:1]
                            )
                            nc.sync.dma_start(y_d.ap()[:, sls[i]], out_t[:])
                        break
                    es = []
                    for i in range(GRP):
                        e = epool.tile([100, T], F32, tag="e")
                        nc.scalar.activation(
                            e[0:out_dim, :], pss[i][0:out_dim, :], ACTF.Exp,
                            bias=bts[l][:, 0:1], scale=1.0,
                        )
                        es.append(e)
                    lts = []
                    for i in range(GRP):
                        lt = lpool.tile([100, T], F32, tag="l")
                        nc.scalar.activation(
                            lt[0:out_dim, :], es[i][0:out_dim, :], ACTF.Ln, bias=1.0
                        )
                        lts.append(lt)
                    nprevs = []
                    for i in range(GRP):
                        m = mpool.tile([100, T], MMDT, tag="m")
                        if l == 3:
                            nc.sync.dma_start(
                                m[98:100, :], xt_d.ap()[:, sls[i]].bitcast(MMDT)
                            )
                        nc.vector._custom_dve(
                            sp_fin,
                            out=m[0:out_dim, :],
                            in0=lts[i][0:out_dim, :],
                            in1=pss[i][0:out_dim, :],
                            s0=bts[l][:, 0:1],
                            s1=40.0,
                        )
                        nprevs.append(m)
                    prevs = nprevs

    nc.compile()
    nc.m = get_hw_module(nc.m)
    return nc


def _transform_weights(inputs):
    """Host-side weight/bias transform -> per-program DRAM tensors (shared
    across cores)."""
    W = [np.asarray(inputs[f"W{l}"], dtype=np.float32) for l in range(9)]
    b = [np.asarray(inputs[f"b{l}"], dtype=np.float32) for l in range(9)]
    t = {}
    t["lhsT0"] = np.ascontiguousarray((100.0 * W[0]).T)
    for l in (1, 2, 3, 5, 6, 7):
        t[f"lhsT{l}"] = np.ascontiguousarray(W[l].T)
    t["lhsT4"] = np.ascontiguousarray(
        np.concatenate([W[4][:, 2:].T, (100.0 * W[4][:, :2]).T], axis=0)
    )
    t["lhsT8"] = np.ascontiguousarray(W[8].T / 100.0)
    for l in range(8):
        t[f"bias{l}"] = np.ascontiguousarray((100.0 * b[l]).reshape(-1, 1))
    t["b8"] = np.ascontiguousarray(b[8].reshape(1, 1))
    return t


_NC_CACHE = None


def kernel(**inputs) -> np.ndarray:
    global _NC_CACHE
    if _NC_CACHE is None:
        _NC_CACHE = _build_program()
    nc = _NC_CACHE

    x = np.asarray(inputs["input"], dtype=np.float32)
    assert x.shape == (N_TOTAL, 2)
    shared = _transform_weights(inputs)

    in_maps = []
    for c in range(N_CORES):
        m = dict(shared)
        m["xt"] = np.ascontiguousarray(x[c * P : (c + 1) * P].T)
        in_maps.append(m)

    res = bass_utils.run_bass_kernel_spmd(nc, in_maps, core_ids=list(range(N_CORES)))
    y = np.concatenate([res.results[c]["y"][0] for c in range(N_CORES)])
    return y.reshape(N_TOTAL, 1).astype(np.float32)
